# revision 3
# baseline (speedup 1.0000x reference)
"""Multi-head SAGE layer (mean aggregation) as a Bass/Tile kernel on 8 trn2 cores.

Math: out = mean_h( h @ W_self[h] + segmean(h[src] by dst) @ W_neigh[h] + b[h] )
    = h @ mean_h(W_self) + segmean(h[src] by dst) @ mean_h(W_neigh) + mean_h(b)
(mean over heads commutes with the linear layers).

Sharding: nodes (and their incident edges, keyed by dst) are split across the
8 cores, 12500 nodes each; h is replicated per-core (bf16 copy) as the gather
table.  Edges are grouped by (src-range bucket, 128-node dst block), padded
per group to T[b,k]*128 slots (T = max count over the 8 cores so the SPMD
schedule is shared), and the per-bucket slot streams are packed into gather
windows of up to ~4K slots.  One dma_gather call fetches a whole window
(the SWDGE descriptor ring is enlarged to 4096 descriptors per queue so a
window's descriptors fit), cutting the per-call GPSIMD fixed cost ~6x vs
one call per (block, bucket).

The segment-sum runs on the tensor engine in bf16, directly in transposed
orientation: per 128-edge sub-tile, matmul(lhsT=E, rhs=msel) accumulates
S^T[feat, node] in PSUM, where msel[e, n] = (dst_local[e] == n) is built for
ALL of a block's sub-tiles with a single wide vector is_equal (stride-0
broadcast AP).  S^T feeds the W_neigh matmul without any transpose; the
1/deg normalization is applied to the matmul OUTPUT (diagonal scaling
commutes), and the self term + bias accumulate in a second PSUM tile.

All graph-structure preprocessing (edge partition/sort/padding, degree
counts, layout transposes, bf16 casts) happens on the host; all matrix math
on the features/weights happens on-device.
"""

import sys

import ml_dtypes
import numpy as np

for _p in ("/opt/trn_rl_repo",):
    if _p not in sys.path:
        sys.path.insert(0, _p)

BF16 = ml_dtypes.bfloat16

N_NODES = 100000
N_EDGES = 1600000
D = 128
H = 4
N_CORES = 8
P = 128
NPC = N_NODES // N_CORES          # nodes per core
NB = (NPC + P - 1) // P           # 128-node blocks per core
NPAD = NB * P                     # padded nodes per core
NBUCKETS = 4
BUCKET_SZ = -(-N_NODES // NBUCKETS)   # src rows per gather bucket (int16 range)
WMAX = 8 * P                      # max gather-window slots (per-call limit)
RING = 65536                      # dynamic_dma_scratch_size -> 4096 desc/queue


def _preprocess(src, dst):
    """Partition edges by dst owner core, group by (128-node dst block, src
    bucket), sort by src within each group, and pad each (b, k) group to
    R[b,k] = T[b,k]*128 gather slots (T = max count over the 8 cores, so the
    SPMD schedule is shared; pad slots gather row 0 of the bucket and are
    masked out of the segment-sum by rseg = -1).

    Per bucket, the R-padded groups are packed in block order into windows of
    at most WMAX slots; each window is one dma_gather call.

    Returns (per_core, sched) where per_core[c]:
      idx16 [128, sumW16] int16  bucket-local gather indices, wrapped-16
                                 layout, windows in (k, w) order; pad = 0
      rseg  [128, sumT] bf16     dst id local to the block in [0,128),
                                 pad = -1; column order (b, k, t)
      deg   [128, NB]   int32    in-degree per node, column b = block b
    and sched has the shared static schedule (T, col_off, windows, ...).
    """
    deg_full = np.bincount(dst, minlength=N_NODES).astype(np.int32)
    counts = np.zeros((N_CORES, NB, NBUCKETS), np.int64)
    blocks = []
    for c in range(N_CORES):
        lo = c * NPC
        m = (dst >= lo) & (dst < lo + NPC)
        s_c = src[m].astype(np.int64)
        d_c = (dst[m] - lo).astype(np.int64)
        blk = d_c >> 7
        bkt = s_c // BUCKET_SZ
        order = np.lexsort((s_c, bkt, blk))
        s_c, d_c, blk, bkt = s_c[order], d_c[order], blk[order], bkt[order]
        key = (blk * NBUCKETS + bkt)
        bc = np.bincount(key, minlength=NB * NBUCKETS)
        counts[c] = bc.reshape(NB, NBUCKETS)
        off = np.zeros(NB * NBUCKETS + 1, np.int64)
        np.cumsum(bc, out=off[1:])
        blocks.append((s_c, d_c, off))

    cmax = counts.max(axis=0)                            # [NB, NBUCKETS]
    T = (-(-np.maximum(1, cmax) // P)).astype(np.int64)  # matmul sub-tiles
    R = T * P                                            # gather slots

    # rseg/msel column offsets, (b, k, t) order (block-major)
    col_off = np.zeros((NB, NBUCKETS), np.int64)
    acc_t = 0
    for b in range(NB):
        for k in range(NBUCKETS):
            col_off[b, k] = acc_t
            acc_t += T[b, k]
    sumT = int(acc_t)
    nmm = T.sum(axis=1).astype(np.int64)                 # matmuls per block
    nmmax = int(nmm.max())

    # gather windows, per bucket (bucket-major streams)
    # windows[k] = list of dicts(b0, b1, slots, idx_off)
    # sub_off[b, k] = (window idx, slot offset within window)
    windows = [[] for _ in range(NBUCKETS)]
    sub_off = np.zeros((NB, NBUCKETS, 2), np.int64)
    idx_acc = 0
    for k in range(NBUCKETS):
        w = {"b0": 0, "slots": 0, "idx_off": idx_acc}
        for b in range(NB):
            r = int(R[b, k])
            if w["slots"] + r > WMAX:
                w["b1"] = b
                windows[k].append(w)
                idx_acc += w["slots"] // 16
                w = {"b0": b, "slots": 0, "idx_off": idx_acc}
            sub_off[b, k] = (len(windows[k]), w["slots"])
            w["slots"] += r
        w["b1"] = NB
        windows[k].append(w)
        idx_acc += w["slots"] // 16
    sumW16 = int(idx_acc)

    per_core = []
    for c in range(N_CORES):
        s_c, d_c, off = blocks[c]
        idx_cols = np.zeros((16, sumW16), np.int16)
        rseg_flat = np.full(sumT * P, -1.0, np.float32)
        for b in range(NB):
            for k in range(NBUCKETS):
                n = int(counts[c, b, k])
                if n == 0:
                    continue
                o = int(off[b * NBUCKETS + k])
                # gather indices: window idx_off + in-window slot offset
                wi, so = sub_off[b, k]
                w = windows[k][wi]
                flat = np.zeros(int(R[b, k]), np.int64)
                flat[:n] = s_c[o:o + n] - k * BUCKET_SZ
                w0 = w["idx_off"] + int(so) // 16
                idx_cols[:, w0:w0 + int(R[b, k]) // 16] = flat.reshape(
                    -1, 16).T
                # rseg (block-major columns)
                base = int(col_off[b, k]) * P
                rseg_flat[base:base + n] = (d_c[o:o + n] - b * P).astype(
                    np.float32)
        idx16 = np.tile(np.ascontiguousarray(idx_cols), (8, 1))
        rseg_t = np.ascontiguousarray(
            rseg_flat.reshape(sumT, P).T).astype(BF16)
        degc = np.zeros(NPAD, np.int32)
        degc[:NPC] = deg_full[c * NPC:(c + 1) * NPC]
        deg_t = np.ascontiguousarray(degc.reshape(NB, P).T)
        per_core.append({"idx16": idx16, "rseg": rseg_t, "deg": deg_t})

    sched = {
        "T": T, "R": R, "col_off": col_off, "windows": windows,
        "sub_off": sub_off, "sumT": sumT, "sumW16": sumW16,
        "nmm": nmm, "nmmax": nmmax,
    }
    return per_core, sched


def build_program(sched, n_nodes=N_NODES, nb=NB, npad=NPAD):
    """Trace + compile the SPMD Bass program for the given group schedule."""
    from contextlib import ExitStack

    from concourse import bacc, mybir, tile

    f32 = mybir.dt.float32
    bf16 = mybir.dt.bfloat16
    i32 = mybir.dt.int32
    i16 = mybir.dt.int16
    AL = mybir.AluOpType

    T = sched["T"]
    col_off = sched["col_off"]
    windows = sched["windows"]
    sub_off = sched["sub_off"]
    sumT = sched["sumT"]
    sumW16 = sched["sumW16"]
    nmm = sched["nmm"]
    nmmax = sched["nmmax"]
    wslots_max = max(w["slots"] for ws in windows for w in ws)

    nc = bacc.Bacc("TRN2", target_bir_lowering=False, debug=False,
                   num_devices=N_CORES, num_swdge_queues=NBUCKETS,
                   dynamic_dma_scratch_size=RING)
    h_ap = nc.dram_tensor("h16", [n_nodes, D], bf16, kind="ExternalInput").ap()
    hT_ap = nc.dram_tensor("hT16", [P, npad], bf16, kind="ExternalInput").ap()
    idx_ap = nc.dram_tensor("idx16", [P, sumW16], i16,
                            kind="ExternalInput").ap()
    rseg_ap = nc.dram_tensor("rseg", [P, sumT], bf16,
                             kind="ExternalInput").ap()
    deg_ap = nc.dram_tensor("deg", [P, nb], i32, kind="ExternalInput").ap()
    iota_ap = nc.dram_tensor("iotaw", [P, nmmax * P], bf16,
                             kind="ExternalInput").ap()
    ws_ap = nc.dram_tensor("W_self", [H, D, D], f32, kind="ExternalInput").ap()
    wn_ap = nc.dram_tensor("W_neigh", [H, D, D], f32,
                           kind="ExternalInput").ap()
    b_ap = nc.dram_tensor("b", [H, D], f32, kind="ExternalInput").ap()
    out_ap = nc.dram_tensor("out", [npad, D], f32, kind="ExternalOutput").ap()

    bucket_aps = []
    for k in range(NBUCKETS):
        lo = k * BUCKET_SZ
        hi = min(n_nodes, lo + BUCKET_SZ)
        bucket_aps.append(h_ap[lo:hi, :])

    with tile.TileContext(nc) as tc, ExitStack() as ctx:
        const = ctx.enter_context(tc.tile_pool(name="const", bufs=1))
        eps = [ctx.enter_context(tc.tile_pool(name=f"eg{k}", bufs=2))
               for k in range(NBUCKETS)]
        mp = ctx.enter_context(tc.tile_pool(name="msel", bufs=2))
        sp = ctx.enter_context(tc.tile_pool(name="small", bufs=3))
        pseg = ctx.enter_context(tc.tile_pool(name="pseg", bufs=2,
                                              space="PSUM"))
        pout = ctx.enter_context(tc.tile_pool(name="pout", bufs=2,
                                              space="PSUM"))
        pslf = ctx.enter_context(tc.tile_pool(name="pslf", bufs=2,
                                              space="PSUM"))

        # ---- prologue: constants ----
        iotaw = const.tile([P, nmmax * P], bf16, tag="iotaw")
        nc.sync.dma_start(iotaw[:], iota_ap)

        # head-averaged weights: wm = 0.25 * sum_h W[h], cast to bf16
        wmeans = []
        for name, ap in (("ws", ws_ap), ("wn", wn_ap)):
            heads = []
            for hh in range(H):
                t = const.tile([P, P], f32, tag=f"{name}h{hh}")
                nc.sync.dma_start(t[:], ap[hh])
                heads.append(t)
            s01 = const.tile([P, P], f32, tag=f"{name}s01")
            nc.vector.tensor_tensor(s01[:], heads[0][:], heads[1][:],
                                    op=AL.add)
            s23 = const.tile([P, P], f32, tag=f"{name}s23")
            nc.vector.tensor_tensor(s23[:], heads[2][:], heads[3][:],
                                    op=AL.add)
            s = const.tile([P, P], f32, tag=f"{name}sum")
            nc.vector.tensor_tensor(s[:], s01[:], s23[:], op=AL.add)
            wm = const.tile([P, P], bf16, tag=f"{name}m")
            nc.scalar.mul(wm[:], s[:], 1.0 / H)
            wmeans.append(wm)
        wsm, wnm = wmeans

        # bias matmul operands: q[h, m] = 1/H; pself += q.T @ b_sb
        b_sb = const.tile([H, P], f32, tag="bsb")
        nc.sync.dma_start(b_sb[:], b_ap)
        b16 = const.tile([H, P], bf16, tag="b16")
        nc.vector.tensor_copy(b16[:], b_sb[:])
        q16 = const.tile([H, P], bf16, tag="q16")
        nc.vector.memset(q16[:], 1.0 / H)

        # inverse degree: 1 / max(deg, 1)
        degsb = const.tile([P, nb], i32, tag="degsb")
        nc.sync.dma_start(degsb[:], deg_ap)
        degf = const.tile([P, nb], f32, tag="degf")
        nc.vector.tensor_copy(degf[:], degsb[:])
        nc.vector.tensor_scalar_max(degf[:], degf[:], 1.0)
        invd = const.tile([P, nb], f32, tag="invd")
        nc.vector.reciprocal(invd[:], degf[:])

        # edge structure, resident in SBUF
        idx_all = const.tile([P, sumW16], i16, tag="idx_all")
        nc.sync.dma_start(idx_all[:], idx_ap)
        rseg_all = const.tile([P, sumT], bf16, tag="rseg_all")
        nc.sync.dma_start(rseg_all[:], rseg_ap)

        etiles = [None] * NBUCKETS

        def issue_window(k, wi):
            w = windows[k][wi]
            slots = w["slots"]
            E = eps[k].tile([P, wslots_max], bf16, tag=f"E{k}")
            nc.gpsimd.dma_gather(
                E[:, :slots].rearrange("p (c d) -> p c d", d=D),
                bucket_aps[k],
                idx_all[:, w["idx_off"]:w["idx_off"] + slots // 16],
                slots,
                slots,
                D,
                queue_num=k,
            )
            etiles[k] = E

        for k in range(NBUCKETS):
            issue_window(k, 0)
        cur_win = [0] * NBUCKETS
        cur_E = list(etiles)

        # ---- main loop over 128-node dst blocks ----
        for b in range(nb):
            # advance / prefetch windows
            for k in range(NBUCKETS):
                wi = int(sub_off[b, k, 0])
                if wi != cur_win[k]:
                    cur_win[k] = wi
                    cur_E[k] = etiles[k]
                if windows[k][wi]["b0"] == b and wi + 1 < len(windows[k]):
                    issue_window(k, wi + 1)

            nb_mm = int(nmm[b])
            c0 = int(col_off[b, 0])

            # wide msel: one is_equal over all of this block's sub-tiles
            msel = mp.tile([P, nmmax * P], bf16, tag="msel")
            nc.vector.tensor_tensor(
                out=msel[:, :nb_mm * P].rearrange("p (c d) -> p c d", d=P),
                in0=rseg_all[:, c0:c0 + nb_mm].to_broadcast([P, nb_mm, P]),
                in1=iotaw[:, :nb_mm * P].rearrange("p (c d) -> p c d", d=P),
                op=AL.is_equal,
            )

            # segment-sum, transposed: psT[feat, node] += E_t.T @ msel_t
            ps = pseg.tile([P, P], f32, tag="seg")
            i = 0
            for k in range(NBUCKETS):
                Tk = int(T[b, k])
                so = int(sub_off[b, k, 1])
                mo = int(col_off[b, k]) - c0
                for t in range(Tk):
                    nc.tensor.matmul(
                        ps[:],
                        lhsT=cur_E[k][:, (so + t * P):(so + (t + 1) * P)],
                        rhs=msel[:, (mo + t) * P:(mo + t + 1) * P],
                        start=(i == 0),
                        stop=(i == nb_mm - 1),
                    )
                    i += 1

            # S^T to SBUF in bf16 (ACT engine)
            psb = sp.tile([P, P], bf16, tag="psb")
            nc.scalar.copy(psb[:], ps[:])

            # neighbor term: po[node, dout] = S.T.T @ wnm  (un-normalized)
            po = pout.tile([P, P], f32, tag="po")
            nc.tensor.matmul(po[:], lhsT=psb[:], rhs=wnm[:],
                             start=True, stop=True)

            # self term + bias: pself[node, dout] = h_blk @ wsm + 1/H sum b
            hTt = sp.tile([P, P], bf16, tag="hTt")
            nc.sync.dma_start(hTt[:], hT_ap[:, b * P:(b + 1) * P])
            pf = pslf.tile([P, P], f32, tag="pself")
            nc.tensor.matmul(pf[:], lhsT=hTt[:], rhs=wsm[:],
                             start=True, stop=False)
            nc.tensor.matmul(pf[:], lhsT=q16[:], rhs=b16[:],
                             start=False, stop=True)

            # out = po * invd[node] + pself
            ot = sp.tile([P, P], f32, tag="ot")
            nc.vector.tensor_scalar(out=ot[:], in0=po[:],
                                    scalar1=invd[:, b:b + 1], scalar2=None,
                                    op0=AL.mult)
            ob = sp.tile([P, P], f32, tag="ob")
            nc.vector.tensor_tensor(out=ob[:], in0=ot[:], in1=pf[:],
                                    op=AL.add)
            nc.sync.dma_start(out_ap[b * P:(b + 1) * P, :], ob[:])

    nc.compile()
    return nc


_CACHE = {}


def kernel(h, src, dst, W_self, W_neigh, b):
    return run(h, src, dst, W_self, W_neigh, b)[0]


def run(h, src, dst, W_self, W_neigh, b, trace=False, **kw):
    from concourse.bass_utils import run_bass_kernel_spmd

    h = np.ascontiguousarray(np.asarray(h, dtype=np.float32))
    src = np.asarray(src, dtype=np.int32)
    dst = np.asarray(dst, dtype=np.int32)
    W_self = np.ascontiguousarray(np.asarray(W_self, dtype=np.float32))
    W_neigh = np.ascontiguousarray(np.asarray(W_neigh, dtype=np.float32))
    b = np.ascontiguousarray(np.asarray(b, dtype=np.float32))

    per_core, sched = _preprocess(src, dst)

    key = (tuple(sched["R"].ravel().tolist()),)
    if key not in _CACHE:
        _CACHE[key] = build_program(sched)
    nc = _CACHE[key]

    nmmax = sched["nmmax"]
    iotaw = np.ascontiguousarray(
        np.tile(np.arange(P, dtype=np.float32), (P, nmmax))).astype(BF16)
    h16 = h.astype(BF16)
    in_maps = []
    for c in range(N_CORES):
        hTc = np.zeros((P, NPAD), np.float32)
        hTc[:, :NPC] = h[c * NPC:(c + 1) * NPC].T
        in_maps.append({
            "h16": h16,
            "hT16": np.ascontiguousarray(hTc).astype(BF16),
            "idx16": per_core[c]["idx16"],
            "rseg": per_core[c]["rseg"],
            "deg": per_core[c]["deg"],
            "iotaw": iotaw,
            "W_self": W_self,
            "W_neigh": W_neigh,
            "b": b,
        })

    res = run_bass_kernel_spmd(nc, in_maps, core_ids=list(range(N_CORES)),
                               trace=trace, **kw)
    out = np.concatenate([res.results[c]["out"][:NPC]
                          for c in range(N_CORES)], axis=0)
    return out, res
